# revision 14
# baseline (speedup 1.0000x reference)
"""Trainium2 Bass kernel for a CQT (constant-Q transform) nn.Module.

Reference computation (per batch sample b, channel c):
    out[b, c, k, f, 0] = sum_t x[b, c, f*HOP + t] * w_re[k, t]
    out[b, c, k, f, 1] = sum_t x[b, c, f*HOP + t] * w_im[k, t]
where w_re/w_im are Hann-windowed complex exponentials with per-bin ragged
lengths (longest 11340 samples), HOP=512, 84 bins, 409 frames.

Strategy: data-parallel over the batch (1 sample per NeuronCore, 8 cores).
Per core the correlation is a banded matmul over 128-sample contraction
chunks; chunk c of every bin uses the same moving slice of the resident
signal tile X[r, rc, m] = x[m*512 + rc*128 + r].

Precision/perf scheme: fp8(e4m3) matmuls in DoubleRow perf mode, which
contracts TWO adjacent 128-chunks per instruction (lhsT [128,2,M],
rhs [128,2,F]) at half the per-row cost of fp16. Accuracy is recovered
with a graded residual expansion per chunk-pair, using x = X8 + Xr and
w = W8 + Wr (each term quantized to e4m3):
  high-energy pairs       (0..17): W8·X8 + W8·Xr + Wr·X8   (err ~1e-3)
  mid pairs          (18..24, G2): W8·X8 + W8·Xr           (err ~1e-2)
  low-energy tail pairs  (25..41): W8·X8                   (err ~8e-3)
The march is trimmed from 89 to 84 chunks (bin 0 loses its last 588
window samples, err ~1e-3) so it splits into 42 even pairs; bins 64..83
(rows 128..167) form a separate 40-row group G2 trimmed to one pair.
The dual-fp8 weight load requires the stride between the two weight
planes to be 16-byte aligned, hence the padded plane widths.
Measured end-to-end relative error vs the fp32 reference: 1.31e-2
(gate: 2e-2), bit-identical across runs. Modeled exec: 21161 ns
(baseline 40394 ns).

Weight rows are interleaved (re_k, im_k) pairs: rows 0..127 = bins 0..63
accumulate in one PSUM bank per channel, rows 128..167 = bins 64..83 in a
second bank. Row bands are copied to SBUF and DMA'd out mid-stream as the
ragged march retires them (pairs only touch a shrinking row prefix), so
only a 16-row copy + DMA trails the last matmul.
"""

import math
from contextlib import ExitStack

import ml_dtypes
import numpy as np

import concourse.bass as bass
import concourse.mybir as mybir
import concourse.tile as tile
from concourse import bacc
from concourse.bass_utils import run_bass_kernel_spmd

# ---- problem constants (hardcoded CQT spec) ----
SR = 22050
N_BINS = 84
BPO = 12
FMIN = 32.7
HOP = 512
B, C, T = 8, 2, 220500
N_CORES = 8

LMAX = 11340           # longest window
F = 409                # frames: 1 + (T - LMAX)//HOP
MBLK = 432             # 512-sample blocks of x resident per channel
NROWS = 2 * N_BINS     # interleaved (re, im) weight rows
G1ROWS = 128           # rows 0..127 = bins 0..63
G2ROWS = NROWS - G1ROWS  # 40 rows = bins 64..83

M_MARCH = 84           # G1 contraction chunks kept (trimmed from 89)
NPAIR = M_MARCH // 2   # DoubleRow chunk pairs
TERM3 = 18             # pairs < TERM3: 3-term residual
TERM2 = 25             # pairs < TERM2: >= 2-term; rest 1-term
G2TERM = 2             # G2 pair terms
G2C = 2                # G2 chunks kept (trimmed from 3) = one pair
# G1 pairs are split across three PSUM banks by epoch so a finished
# bank can be read out mid-stream without any write-after-read hazard
# against later matmuls (Tile tracks deps at tile granularity, and the
# in-order DVE queue would otherwise stall the PE behind those reads).
# Epoch banks: A = pairs 0..6 (rows 64:128 final), B = 7..17 (32:64
# final), C = 18..41. Row bands then retire as: copy A after pair 6,
# o1 += B bands after pair 17, o1[0:32] += C at the end.
EPOCH_B = 7            # first pair accumulating into bank B
EPOCH_C = 18           # first pair accumulating into bank C

F8 = mybir.dt.float8e4   # e4m3
NP8 = ml_dtypes.float8_e4m3

_PREP = None
_NC = None
LAST_RESULTS = None


def _params():
    """Host-side constants: pair geometry + fp8 weight planes (W8, Wr)."""
    global _PREP
    if _PREP is not None:
        return _PREP

    Q = 1.0 / (2.0 ** (1.0 / BPO) - 1.0)
    freqs = FMIN * 2.0 ** (np.arange(N_BINS, dtype=np.float64) / BPO)
    lengths = np.round(Q * SR / freqs).astype(np.int64)
    assert int(lengths.max()) == LMAX

    t = np.arange(LMAX, dtype=np.float64)
    L = lengths.astype(np.float64)[:, None]
    mask = (t[None, :] < L).astype(np.float64)
    win = 0.5 * (1.0 - np.cos(2.0 * math.pi * t[None, :] / L)) * mask
    phase = (2.0 * math.pi / SR) * freqs[:, None] * t[None, :]
    w_re = win * np.cos(phase)
    w_im = -win * np.sin(phase)

    # rows 2k / 2k+1 = re_k / im_k, zero-padded past LMAX
    W = np.zeros((NROWS, M_MARCH * 128))
    n = min(LMAX, M_MARCH * 128)
    W[0::2, :n] = w_re[:, :n]
    W[1::2, :n] = w_im[:, :n]
    Wg1, Wg2 = W[:G1ROWS], W[G1ROWS:]

    # active rows per chunk (after trim) -> rows per pair = first chunk's
    Ck = np.minimum(np.ceil(lengths / 128).astype(np.int64), M_MARCH)
    n_act = np.array([(Ck[:64] > c).sum() for c in range(M_MARCH)])
    m_c = np.minimum(G1ROWS, 2 * n_act)
    m_p = m_c[0::2].astype(np.int64)
    assert m_p[0] == G1ROWS and (m_c[1::2] <= m_p).all()

    def q8(a):
        return a.astype(NP8).astype(np.float64)

    # w8 column layout: [G2 pair (40) | pair0 | pair1 | ...]
    off8 = np.zeros(NPAIR + 1, dtype=np.int64)
    off8[0] = G2ROWS
    off8[1:] = G2ROWS + np.cumsum(m_p)
    # dual-fp8 Ldweights requires the stride between the two weight
    # planes (the tile's inner width) to be 16-byte aligned
    CW8 = (int(off8[-1]) + 15) & ~15
    offr = np.zeros(TERM3 + 1, dtype=np.int64)
    offr[0] = G2ROWS
    offr[1:] = G2ROWS + np.cumsum(m_p[:TERM3])
    CWr = (int(offr[-1]) + 15) & ~15

    w8 = np.zeros((128, 2, CW8), dtype=np.float64)
    wr8 = np.zeros((128, 2, CWr), dtype=np.float64)
    for i in range(2):
        # G2 pair = chunks 0,1
        w8[:, i, :G2ROWS] = Wg2[:, 128 * i:128 * (i + 1)].T
        for p in range(NPAIR):
            m = int(m_p[p])
            blk = Wg1[:m, 128 * (2 * p + i):128 * (2 * p + i + 1)].T
            w8[:, i, off8[p]:off8[p] + m] = blk
    w8q = q8(w8)
    wr8[:, :, :CWr] = w8[:, :, :CWr] - w8q[:, :, :CWr]

    _PREP = dict(m_p=m_p, off8=off8, offr=offr, CW8=CW8, CWr=CWr,
                 w8=w8q.astype(NP8), wr8=q8(wr8).astype(NP8))
    return _PREP


def _build_nc():
    p = _params()
    m_p, off8, offr = p["m_p"], p["off8"], p["offr"]
    CW8, CWr = p["CW8"], p["CWr"]

    nc = bacc.Bacc(None, target_bir_lowering=False)
    x8_d = nc.dram_tensor("x8", (C, 128, 4, MBLK), F8, kind="ExternalInput")
    xr8_d = nc.dram_tensor("xr8", (C, 128, 4, MBLK), F8, kind="ExternalInput")
    w8_d = nc.dram_tensor("w8", (128, 2, CW8), F8, kind="ExternalInput")
    wr8_d = nc.dram_tensor("wr8", (128, 2, CWr), F8, kind="ExternalInput")
    out_d = nc.dram_tensor("out", (C, NROWS, F), mybir.dt.float32,
                           kind="ExternalOutput")

    with ExitStack() as ctx:
        tc = ctx.enter_context(tile.TileContext(nc))
        xp = ctx.enter_context(tc.tile_pool(name="xp", bufs=1))
        wp = ctx.enter_context(tc.tile_pool(name="wp", bufs=1))
        op = ctx.enter_context(tc.tile_pool(name="op", bufs=1))
        pp = ctx.enter_context(tc.tile_pool(name="pp", bufs=1, space="PSUM"))

        x8_sb = {ch: xp.tile([128, 4, MBLK], F8, name=f"x8_{ch}",
                             tag=f"x8_{ch}") for ch in range(C)}
        xr8_sb = {ch: xp.tile([128, 4, MBLK], F8, name=f"xr8_{ch}",
                              tag=f"xr8_{ch}") for ch in range(C)}
        w8_sb = wp.tile([128, 2, CW8], F8, name="w8_sb", tag="w8")
        wr8_sb = wp.tile([128, 2, CWr], F8, name="wr8_sb", tag="wr8")
        o1 = {ch: op.tile([128, F], mybir.dt.float32, name=f"o1_{ch}",
                          tag=f"o1_{ch}") for ch in range(C)}
        o2 = {ch: op.tile([G2ROWS, F], mybir.dt.float32, name=f"o2_{ch}",
                          tag=f"o2_{ch}") for ch in range(C)}
        # 4 PSUM banks per channel: G1 epochs A/B/C + G2
        psA = {ch: pp.tile([128, 512], mybir.dt.float32, name=f"psA_{ch}",
                           tag=f"psA_{ch}") for ch in range(C)}
        psB = {ch: pp.tile([128, 512], mybir.dt.float32, name=f"psB_{ch}",
                           tag=f"psB_{ch}") for ch in range(C)}
        psC = {ch: pp.tile([128, 512], mybir.dt.float32, name=f"psC_{ch}",
                           tag=f"psC_{ch}") for ch in range(C)}
        ps2 = {ch: pp.tile([128, 512], mybir.dt.float32, name=f"ps2_{ch}",
                           tag=f"ps2_{ch}") for ch in range(C)}

        # --- input DMA plan ---
        # sync (SP, HWDGE): signal, in consumption order. rc01 feeds the
        # even chunk pairs (rc in {0,1}), rc23 the odd ones.
        nc.sync.dma_start(x8_sb[0][:, 0:2, :], x8_d[0, :, 0:2, :])
        nc.sync.dma_start(xr8_sb[0][:, 0:2, :], xr8_d[0, :, 0:2, :])
        nc.sync.dma_start(x8_sb[0][:, 2:4, :], x8_d[0, :, 2:4, :])
        nc.sync.dma_start(xr8_sb[0][:, 2:4, :], xr8_d[0, :, 2:4, :])
        nc.sync.dma_start(x8_sb[1][:], x8_d[1])
        nc.sync.dma_start(xr8_sb[1][:], xr8_d[1])
        # scalar (Act, HWDGE): weights in consumption order, split so the
        # first matmul is gated by a small piece (G2 + pairs 0..1).
        c1 = int(off8[2])
        c2 = int(off8[8])
        nc.scalar.dma_start(w8_sb[:, :, 0:c1], w8_d[:, :, 0:c1])
        nc.scalar.dma_start(w8_sb[:, :, c1:c2], w8_d[:, :, c1:c2])
        r1 = int(offr[2])
        nc.scalar.dma_start(wr8_sb[:, :, 0:r1], wr8_d[:, :, 0:r1])
        nc.scalar.dma_start(w8_sb[:, :, c2:CW8], w8_d[:, :, c2:CW8])
        nc.scalar.dma_start(wr8_sb[:, :, r1:CWr], wr8_d[:, :, r1:CWr])

        def emit_pair(ch, ps, rows, o_w8, o_wr, rc, j, nterm, start):
            """One chunk pair: graded-precision DR matmuls into ps[0:rows]."""
            rhs8 = x8_sb[ch][:, rc:rc + 2, j:j + F]
            nc.tensor.matmul(ps[0:rows, 0:F],
                             w8_sb[:, :, o_w8:o_w8 + rows], rhs8,
                             start=start, stop=False,
                             perf_mode=mybir.MatmulPerfMode.DoubleRow,
                             skip_group_check=True)
            if nterm >= 2:
                rhsr = xr8_sb[ch][:, rc:rc + 2, j:j + F]
                nc.tensor.matmul(ps[0:rows, 0:F],
                                 w8_sb[:, :, o_w8:o_w8 + rows], rhsr,
                                 start=False, stop=False,
                                 perf_mode=mybir.MatmulPerfMode.DoubleRow,
                                 skip_group_check=True)
            if nterm >= 3:
                nc.tensor.matmul(ps[0:rows, 0:F],
                                 wr8_sb[:, :, o_wr:o_wr + rows], rhs8,
                                 start=False, stop=False,
                                 perf_mode=mybir.MatmulPerfMode.DoubleRow,
                                 skip_group_check=True)

        def bank(pr, ch):
            if pr < EPOCH_B:
                return psA[ch]
            return psB[ch] if pr < EPOCH_C else psC[ch]

        for ch in range(C):
            for pr in range(NPAIR):
                rc, j = (2 * pr) % 4, pr // 2
                nterm = 3 if pr < TERM3 else (2 if pr < TERM2 else 1)
                emit_pair(ch, bank(pr, ch), int(m_p[pr]), int(off8[pr]),
                          int(offr[pr]) if pr < TERM3 else 0,
                          rc, j, nterm,
                          start=(pr in (0, EPOCH_B, EPOCH_C)))
                if pr == 0:
                    # G2 pair (rows 128..167) rides right behind pair 0 so
                    # its bank retires early
                    emit_pair(ch, ps2[ch], G2ROWS, 0, 0, 0, 0, G2TERM,
                              start=True)
                    nc.vector.tensor_copy(o2[ch][:], ps2[ch][0:G2ROWS, 0:F])
                    nc.gpsimd.dma_start(out_d[ch, G1ROWS:NROWS, :], o2[ch][:])
                elif pr == EPOCH_B - 1:
                    # bank A final: rows 64:128 ship now; 0:64 seed o1
                    nc.vector.tensor_copy(o1[ch][:, :], psA[ch][:, 0:F])
                    nc.gpsimd.dma_start(out_d[ch, 64:128, :], o1[ch][64:128, :])
                elif pr == EPOCH_C - 1:
                    # bank B final: rows 32:64 ship; 0:32 folded into o1
                    nc.vector.tensor_add(o1[ch][32:64, :], o1[ch][32:64, :],
                                         psB[ch][32:64, 0:F])
                    nc.gpsimd.dma_start(out_d[ch, 32:64, :], o1[ch][32:64, :])
                    nc.vector.tensor_add(o1[ch][0:32, :], o1[ch][0:32, :],
                                         psB[ch][0:32, 0:F])
                elif pr == NPAIR - 1:
                    # bank C final: fold and ship rows 0:32
                    nc.vector.tensor_add(o1[ch][0:32, :], o1[ch][0:32, :],
                                         psC[ch][0:32, 0:F])
                    q = nc.sync if ch == 1 else nc.gpsimd
                    q.dma_start(out_d[ch, 0:32, :], o1[ch][0:32, :])
    nc.finalize()
    return nc


def get_nc():
    global _NC
    if _NC is None:
        _NC = _build_nc()
    return _NC


def _pack_x(xb):
    """(C, T) f32 -> fp8 X8/Xr pair, each (C, 128, 4, MBLK) with
    X[ch, r, rc, m] = x[ch, m*512 + rc*128 + r]."""
    xpad = np.zeros((C, MBLK * 512), dtype=np.float32)
    xpad[:, :T] = xb
    xt = np.ascontiguousarray(
        xpad.reshape(C, MBLK, 4, 128).transpose(0, 3, 2, 1))
    x8 = xt.astype(NP8)
    xr8 = (xt - x8.astype(np.float32)).astype(NP8)
    return np.ascontiguousarray(x8), np.ascontiguousarray(xr8)


def kernel(x):
    global LAST_RESULTS
    x = np.asarray(x, dtype=np.float32)
    assert x.shape == (B, C, T)
    p = _params()
    in_maps = []
    for b in range(B):
        x8, xr8 = _pack_x(x[b])
        in_maps.append({"x8": x8, "xr8": xr8, "w8": p["w8"],
                        "wr8": p["wr8"]})
    nc = get_nc()
    res = run_bass_kernel_spmd(nc, in_maps, core_ids=list(range(N_CORES)))
    LAST_RESULTS = res
    out = np.empty((B, C, N_BINS, F, 2), dtype=np.float32)
    for b in range(B):
        raw = np.asarray(res.results[b]["out"])  # (C, NROWS, F)
        out[b] = raw.reshape(C, N_BINS, 2, F).transpose(0, 1, 3, 2)
    return out


# revision 15
# speedup vs baseline: 1.0247x; 1.0247x over previous
"""Trainium2 Bass kernel for a CQT (constant-Q transform) nn.Module.

Reference computation (per batch sample b, channel c):
    out[b, c, k, f, 0] = sum_t x[b, c, f*HOP + t] * w_re[k, t]
    out[b, c, k, f, 1] = sum_t x[b, c, f*HOP + t] * w_im[k, t]
where w_re/w_im are Hann-windowed complex exponentials with per-bin ragged
lengths (longest 11340 samples), HOP=512, 84 bins, 409 frames.

Strategy: data-parallel over the batch (1 sample per NeuronCore, 8 cores).
Per core the correlation is a banded matmul over 128-sample contraction
chunks; chunk c of every bin uses the same moving slice of the resident
signal tile X[r, rc, m] = x[m*512 + rc*128 + r].

Precision/perf scheme: fp8(e4m3) matmuls in DoubleRow perf mode, which
contracts TWO adjacent 128-chunks per instruction (lhsT [128,2,M],
rhs [128,2,F]) at half the per-row cost of fp16. Accuracy is recovered
with a graded residual expansion per chunk-pair, using x = X8 + Xr and
w = W8 + Wr (each term quantized to e4m3):
  high-energy pairs       (0..17): W8·X8 + W8·Xr + Wr·X8   (err ~1e-3)
  mid pairs          (18..24, G2): W8·X8 + W8·Xr           (err ~1e-2)
  low-energy tail pairs  (25..41): W8·X8                   (err ~8e-3)
The march is trimmed from 89 to 84 chunks (bin 0 loses its last 588
window samples, err ~1e-3) so it splits into 42 even pairs; bins 64..83
(rows 128..167) form a separate 40-row group G2 trimmed to one pair.
The dual-fp8 weight load requires the stride between the two weight
planes to be 16-byte aligned, hence the padded plane widths.
Measured end-to-end relative error vs the fp32 reference: 1.31e-2
(gate: 2e-2), bit-identical across runs. Modeled exec: 21161 ns
(baseline 40394 ns).

Weight rows are interleaved (re_k, im_k) pairs: rows 0..127 = bins 0..63
accumulate in one PSUM bank per channel, rows 128..167 = bins 64..83 in a
second bank. Row bands are copied to SBUF and DMA'd out mid-stream as the
ragged march retires them (pairs only touch a shrinking row prefix), so
only a 16-row copy + DMA trails the last matmul.
"""

import math
from contextlib import ExitStack

import ml_dtypes
import numpy as np

import concourse.bass as bass
import concourse.mybir as mybir
import concourse.tile as tile
from concourse import bacc
from concourse.bass_utils import run_bass_kernel_spmd

# ---- problem constants (hardcoded CQT spec) ----
SR = 22050
N_BINS = 84
BPO = 12
FMIN = 32.7
HOP = 512
B, C, T = 8, 2, 220500
N_CORES = 8

LMAX = 11340           # longest window
F = 409                # frames: 1 + (T - LMAX)//HOP
MBLK = 432             # 512-sample blocks of x resident per channel
NROWS = 2 * N_BINS     # interleaved (re, im) weight rows
G1ROWS = 128           # rows 0..127 = bins 0..63
G2ROWS = NROWS - G1ROWS  # 40 rows = bins 64..83

M_MARCH = 84           # G1 contraction chunks kept (trimmed from 89)
NPAIR = M_MARCH // 2   # DoubleRow chunk pairs
TERM3 = 16             # pairs < TERM3: 3-term residual
TERM2 = 24             # pairs < TERM2: >= 2-term; rest 1-term
G2TERM = 2             # G2 pair terms
G2C = 2                # G2 chunks kept (trimmed from 3) = one pair
# G1 pairs are split across three PSUM banks by epoch so a finished
# bank can be read out mid-stream without any write-after-read hazard
# against later matmuls (Tile tracks deps at tile granularity, and the
# in-order DVE queue would otherwise stall the PE behind those reads).
# Epoch banks: A = pairs 0..6 (rows 64:128 final), B = 7..17 (32:64
# final), C = 18..41. Row bands then retire as: copy A after pair 6,
# o1 += B bands after pair 17, o1[0:32] += C at the end.
EPOCH_B = 7            # first pair accumulating into bank B
EPOCH_C = 18           # first pair accumulating into bank C

F8 = mybir.dt.float8e4   # e4m3
NP8 = ml_dtypes.float8_e4m3

_PREP = None
_NC = None
LAST_RESULTS = None


def _params():
    """Host-side constants: pair geometry + fp8 weight planes (W8, Wr)."""
    global _PREP
    if _PREP is not None:
        return _PREP

    Q = 1.0 / (2.0 ** (1.0 / BPO) - 1.0)
    freqs = FMIN * 2.0 ** (np.arange(N_BINS, dtype=np.float64) / BPO)
    lengths = np.round(Q * SR / freqs).astype(np.int64)
    assert int(lengths.max()) == LMAX

    t = np.arange(LMAX, dtype=np.float64)
    L = lengths.astype(np.float64)[:, None]
    mask = (t[None, :] < L).astype(np.float64)
    win = 0.5 * (1.0 - np.cos(2.0 * math.pi * t[None, :] / L)) * mask
    phase = (2.0 * math.pi / SR) * freqs[:, None] * t[None, :]
    w_re = win * np.cos(phase)
    w_im = -win * np.sin(phase)

    # rows 2k / 2k+1 = re_k / im_k, zero-padded past LMAX
    W = np.zeros((NROWS, M_MARCH * 128))
    n = min(LMAX, M_MARCH * 128)
    W[0::2, :n] = w_re[:, :n]
    W[1::2, :n] = w_im[:, :n]
    Wg1, Wg2 = W[:G1ROWS], W[G1ROWS:]

    # active rows per chunk (after trim) -> rows per pair = first chunk's
    Ck = np.minimum(np.ceil(lengths / 128).astype(np.int64), M_MARCH)
    n_act = np.array([(Ck[:64] > c).sum() for c in range(M_MARCH)])
    m_c = np.minimum(G1ROWS, 2 * n_act)
    m_p = m_c[0::2].astype(np.int64)
    assert m_p[0] == G1ROWS and (m_c[1::2] <= m_p).all()

    def q8(a):
        return a.astype(NP8).astype(np.float64)

    # w8 column layout: [G2 pair (40) | pair0 | pair1 | ...]
    off8 = np.zeros(NPAIR + 1, dtype=np.int64)
    off8[0] = G2ROWS
    off8[1:] = G2ROWS + np.cumsum(m_p)
    # dual-fp8 Ldweights requires the stride between the two weight
    # planes (the tile's inner width) to be 16-byte aligned
    CW8 = (int(off8[-1]) + 15) & ~15
    offr = np.zeros(TERM3 + 1, dtype=np.int64)
    offr[0] = G2ROWS
    offr[1:] = G2ROWS + np.cumsum(m_p[:TERM3])
    CWr = (int(offr[-1]) + 15) & ~15

    w8 = np.zeros((128, 2, CW8), dtype=np.float64)
    wr8 = np.zeros((128, 2, CWr), dtype=np.float64)
    for i in range(2):
        # G2 pair = chunks 0,1
        w8[:, i, :G2ROWS] = Wg2[:, 128 * i:128 * (i + 1)].T
        for p in range(NPAIR):
            m = int(m_p[p])
            blk = Wg1[:m, 128 * (2 * p + i):128 * (2 * p + i + 1)].T
            w8[:, i, off8[p]:off8[p] + m] = blk
    w8q = q8(w8)
    wr8[:, :, :CWr] = w8[:, :, :CWr] - w8q[:, :, :CWr]

    _PREP = dict(m_p=m_p, off8=off8, offr=offr, CW8=CW8, CWr=CWr,
                 w8=w8q.astype(NP8), wr8=q8(wr8).astype(NP8))
    return _PREP


def _build_nc():
    p = _params()
    m_p, off8, offr = p["m_p"], p["off8"], p["offr"]
    CW8, CWr = p["CW8"], p["CWr"]

    nc = bacc.Bacc(None, target_bir_lowering=False)
    x8_d = nc.dram_tensor("x8", (C, 128, 4, MBLK), F8, kind="ExternalInput")
    xr8_d = nc.dram_tensor("xr8", (C, 128, 4, MBLK), F8, kind="ExternalInput")
    w8_d = nc.dram_tensor("w8", (128, 2, CW8), F8, kind="ExternalInput")
    wr8_d = nc.dram_tensor("wr8", (128, 2, CWr), F8, kind="ExternalInput")
    out_d = nc.dram_tensor("out", (C, NROWS, F), mybir.dt.float32,
                           kind="ExternalOutput")

    with ExitStack() as ctx:
        tc = ctx.enter_context(tile.TileContext(nc))
        xp = ctx.enter_context(tc.tile_pool(name="xp", bufs=1))
        wp = ctx.enter_context(tc.tile_pool(name="wp", bufs=1))
        op = ctx.enter_context(tc.tile_pool(name="op", bufs=1))
        pp = ctx.enter_context(tc.tile_pool(name="pp", bufs=1, space="PSUM"))

        x8_sb = {ch: xp.tile([128, 4, MBLK], F8, name=f"x8_{ch}",
                             tag=f"x8_{ch}") for ch in range(C)}
        xr8_sb = {ch: xp.tile([128, 4, MBLK], F8, name=f"xr8_{ch}",
                              tag=f"xr8_{ch}") for ch in range(C)}
        w8_sb = wp.tile([128, 2, CW8], F8, name="w8_sb", tag="w8")
        wr8_sb = wp.tile([128, 2, CWr], F8, name="wr8_sb", tag="wr8")
        o1 = {ch: op.tile([128, F], mybir.dt.float32, name=f"o1_{ch}",
                          tag=f"o1_{ch}") for ch in range(C)}
        o2 = {ch: op.tile([G2ROWS, F], mybir.dt.float32, name=f"o2_{ch}",
                          tag=f"o2_{ch}") for ch in range(C)}
        # 4 PSUM banks per channel: G1 epochs A/B/C + G2
        psA = {ch: pp.tile([128, 512], mybir.dt.float32, name=f"psA_{ch}",
                           tag=f"psA_{ch}") for ch in range(C)}
        psB = {ch: pp.tile([128, 512], mybir.dt.float32, name=f"psB_{ch}",
                           tag=f"psB_{ch}") for ch in range(C)}
        psC = {ch: pp.tile([128, 512], mybir.dt.float32, name=f"psC_{ch}",
                           tag=f"psC_{ch}") for ch in range(C)}
        ps2 = {ch: pp.tile([128, 512], mybir.dt.float32, name=f"ps2_{ch}",
                           tag=f"ps2_{ch}") for ch in range(C)}

        # --- input DMA plan ---
        # sync (SP, HWDGE): signal, in consumption order. rc01 feeds the
        # even chunk pairs (rc in {0,1}), rc23 the odd ones.
        nc.sync.dma_start(x8_sb[0][:, 0:2, :], x8_d[0, :, 0:2, :])
        nc.sync.dma_start(xr8_sb[0][:, 0:2, :], xr8_d[0, :, 0:2, :])
        nc.sync.dma_start(x8_sb[0][:, 2:4, :], x8_d[0, :, 2:4, :])
        nc.sync.dma_start(xr8_sb[0][:, 2:4, :], xr8_d[0, :, 2:4, :])
        nc.sync.dma_start(x8_sb[1][:], x8_d[1])
        nc.sync.dma_start(xr8_sb[1][:], xr8_d[1])
        # scalar (Act, HWDGE): weights in consumption order, split so the
        # first matmul is gated by a small piece (G2 + pairs 0..1).
        c1 = int(off8[2])
        c2 = int(off8[8])
        nc.scalar.dma_start(w8_sb[:, :, 0:c1], w8_d[:, :, 0:c1])
        nc.scalar.dma_start(w8_sb[:, :, c1:c2], w8_d[:, :, c1:c2])
        r1 = int(offr[2])
        nc.scalar.dma_start(wr8_sb[:, :, 0:r1], wr8_d[:, :, 0:r1])
        nc.scalar.dma_start(w8_sb[:, :, c2:CW8], w8_d[:, :, c2:CW8])
        nc.scalar.dma_start(wr8_sb[:, :, r1:CWr], wr8_d[:, :, r1:CWr])

        def emit_pair(ch, ps, rows, o_w8, o_wr, rc, j, nterm, start):
            """One chunk pair: graded-precision DR matmuls into ps[0:rows]."""
            rhs8 = x8_sb[ch][:, rc:rc + 2, j:j + F]
            nc.tensor.matmul(ps[0:rows, 0:F],
                             w8_sb[:, :, o_w8:o_w8 + rows], rhs8,
                             start=start, stop=False,
                             perf_mode=mybir.MatmulPerfMode.DoubleRow,
                             skip_group_check=True)
            if nterm >= 2:
                rhsr = xr8_sb[ch][:, rc:rc + 2, j:j + F]
                nc.tensor.matmul(ps[0:rows, 0:F],
                                 w8_sb[:, :, o_w8:o_w8 + rows], rhsr,
                                 start=False, stop=False,
                                 perf_mode=mybir.MatmulPerfMode.DoubleRow,
                                 skip_group_check=True)
            if nterm >= 3:
                nc.tensor.matmul(ps[0:rows, 0:F],
                                 wr8_sb[:, :, o_wr:o_wr + rows], rhs8,
                                 start=False, stop=False,
                                 perf_mode=mybir.MatmulPerfMode.DoubleRow,
                                 skip_group_check=True)

        def bank(pr, ch):
            if pr < EPOCH_B:
                return psA[ch]
            return psB[ch] if pr < EPOCH_C else psC[ch]

        for ch in range(C):
            for pr in range(NPAIR):
                rc, j = (2 * pr) % 4, pr // 2
                nterm = 3 if pr < TERM3 else (2 if pr < TERM2 else 1)
                emit_pair(ch, bank(pr, ch), int(m_p[pr]), int(off8[pr]),
                          int(offr[pr]) if pr < TERM3 else 0,
                          rc, j, nterm,
                          start=(pr in (0, EPOCH_B, EPOCH_C)))
                if pr == 0:
                    # G2 pair (rows 128..167) rides right behind pair 0 so
                    # its bank retires early
                    emit_pair(ch, ps2[ch], G2ROWS, 0, 0, 0, 0, G2TERM,
                              start=True)
                    nc.vector.tensor_copy(o2[ch][:], ps2[ch][0:G2ROWS, 0:F])
                    nc.gpsimd.dma_start(out_d[ch, G1ROWS:NROWS, :], o2[ch][:])
                elif pr == EPOCH_B - 1:
                    # bank A final: rows 64:128 ship now; 0:64 seed o1
                    nc.vector.tensor_copy(o1[ch][:, :], psA[ch][:, 0:F])
                    nc.gpsimd.dma_start(out_d[ch, 64:128, :], o1[ch][64:128, :])
                elif pr == EPOCH_C - 1:
                    # bank B final: rows 32:64 ship; 0:32 folded into o1
                    nc.vector.tensor_add(o1[ch][32:64, :], o1[ch][32:64, :],
                                         psB[ch][32:64, 0:F])
                    nc.gpsimd.dma_start(out_d[ch, 32:64, :], o1[ch][32:64, :])
                    nc.vector.tensor_add(o1[ch][0:32, :], o1[ch][0:32, :],
                                         psB[ch][0:32, 0:F])
                elif pr == NPAIR - 1:
                    # bank C final: fold and ship rows 0:32
                    nc.vector.tensor_add(o1[ch][0:32, :], o1[ch][0:32, :],
                                         psC[ch][0:32, 0:F])
                    q = nc.sync if ch == 1 else nc.gpsimd
                    q.dma_start(out_d[ch, 0:32, :], o1[ch][0:32, :])
    nc.finalize()
    return nc


def get_nc():
    global _NC
    if _NC is None:
        _NC = _build_nc()
    return _NC


def _pack_x(xb):
    """(C, T) f32 -> fp8 X8/Xr pair, each (C, 128, 4, MBLK) with
    X[ch, r, rc, m] = x[ch, m*512 + rc*128 + r]."""
    xpad = np.zeros((C, MBLK * 512), dtype=np.float32)
    xpad[:, :T] = xb
    xt = np.ascontiguousarray(
        xpad.reshape(C, MBLK, 4, 128).transpose(0, 3, 2, 1))
    x8 = xt.astype(NP8)
    xr8 = (xt - x8.astype(np.float32)).astype(NP8)
    return np.ascontiguousarray(x8), np.ascontiguousarray(xr8)


def kernel(x):
    global LAST_RESULTS
    x = np.asarray(x, dtype=np.float32)
    assert x.shape == (B, C, T)
    p = _params()
    in_maps = []
    for b in range(B):
        x8, xr8 = _pack_x(x[b])
        in_maps.append({"x8": x8, "xr8": xr8, "w8": p["w8"],
                        "wr8": p["wr8"]})
    nc = get_nc()
    res = run_bass_kernel_spmd(nc, in_maps, core_ids=list(range(N_CORES)))
    LAST_RESULTS = res
    out = np.empty((B, C, N_BINS, F, 2), dtype=np.float32)
    for b in range(B):
        raw = np.asarray(res.results[b]["out"])  # (C, NROWS, F)
        out[b] = raw.reshape(C, N_BINS, 2, F).transpose(0, 1, 3, 2)
    return out
